# revision 1
# baseline (speedup 1.0000x reference)
"""APoT quantizer (vq_codebook) distributed Bass kernel for 8 TRN2 NeuronCores.

Sharding: data-parallel along dim 0 (4096 rows -> 512 rows/core); alpha-derived
scalars are replicated per-partition via a tiny consts tensor.

Algorithm (signed-mantissa bit tricks): the APoT level set for bits=8, k=2,
signed is exactly the two-hot set {0} U {+-(2^-p + 2^-q)}, so nearest-level
quantization decomposes into float bit ops — no table search:

  ys   = x * (1/alpha)                      [ACT Copy, exact RN mult]
  (y'  = clamp(ys, +-(1-2^-24))             [SAFE variant only])
  base = ys & 0xFF800000                    [sign + exponent = signed po2]
  m    = (ys & 0x7FFFFF) | 0x3F800000       [mantissa in [1,2)]
  r'   = m*C2 - C2                          [= RN((m-1)*C2), exact: ACT affine
                                             is a single-rounded FMA and m-1 is
                                             Sterbenz-exact. C2 = fp32 just
                                             below 4/3 => exponent field of r'
                                             is round-to-nearest-po2 of (m-1)
                                             with ties-to-lower]
  rq   = r' & 0x7F800000
  yq   = (rq + 1) * base                    [DVE fused stt; exact: both stages
                                             fit in 24-bit mantissas]
  out  = yq * alpha                         [ACT Copy FMA = RN(yq*a)]

Matches the reference searchsorted-nearest semantics except (a) |x|*inv_a vs
x/a rounding within 1ulp of a level midpoint (measure-zero), (b) the
reference's absolute 2^-12 floor / 2^-13 zero-snap / tiny-y staircase are
approximated by the relative po2 round (|err| <= ~2^-12*alpha on ~1% of
elements). Norm rel-err on the N(0,1)/alpha=max|x| data: ~1e-4.

The FAST variant (no clamp) requires alpha >= max|x| (guaranteed by the
reference's setup: alpha = |x|.max()); kernel() verifies this on the host
and falls back to the SAFE variant otherwise.
"""
import numpy as np
from contextlib import ExitStack

N_CORES = 8
ROWS, COLS = 4096, 16384
SHARD_ROWS = ROWS // N_CORES  # 512
P = 128
PB = SHARD_ROWS // P          # partition blocks per core

EPS = 1e-8
C2 = float(np.uint32(0x3FAAAAAA).view(np.float32))          # just below 4/3
CLAMP_HI = float(np.uint32(0x3F7FFFFF).view(np.float32))    # 1 - 2^-24

_CACHE = {}


def _build(variant="fast", reps=1, fd=2048, tmp_bufs=3, io_bufs=3,
           r_dve_every=10 ** 9, tail="stt", share=0, chain_bufs=5):
    import concourse.tile as tile
    from concourse import bacc, mybir

    F32 = mybir.dt.float32
    U32 = mybir.dt.uint32
    ALU = mybir.AluOpType
    ACTF = mybir.ActivationFunctionType

    nc = bacc.Bacc("TRN2", target_bir_lowering=False, debug=False,
                   num_devices=N_CORES)
    x_d = nc.dram_tensor("x", [SHARD_ROWS, COLS], F32, kind="ExternalInput")
    c_d = nc.dram_tensor("consts", [P, 8], F32, kind="ExternalInput")
    o_d = nc.dram_tensor("out", [SHARD_ROWS, COLS], F32, kind="ExternalOutput")

    nt = COLS // fd
    with tile.TileContext(nc) as tc, ExitStack() as ctx:
        cpool = ctx.enter_context(tc.tile_pool(name="cp", bufs=1))
        io_pool = ctx.enter_context(tc.tile_pool(name="io", bufs=io_bufs))
        tmp = ctx.enter_context(tc.tile_pool(name="tmp", bufs=tmp_bufs))
        chain = (ctx.enter_context(tc.tile_pool(name="chain",
                                                bufs=chain_bufs))
                 if chain_bufs else tmp)

        consts = cpool.tile([P, 8], F32)
        nc.sync.dma_start(consts[:], c_d[:])
        a_ap = consts[:, 0:1]
        ia_ap = consts[:, 1:2]

        idx = 0
        for rep in range(reps):
            for blk in range(PB):
                rows = slice(blk * P, (blk + 1) * P)
                for j in range(nt):
                    cols = slice(j * fd, (j + 1) * fd)
                    t_x = io_pool.tile([P, fd], F32, tag="t_x")
                    nc.sync.dma_start(t_x[:], x_d[rows, cols])

                    t_y = chain.tile([P, fd], F32, tag="t_y")
                    t_b = tmp.tile([P, fd], F32, tag="t_b")
                    t_m = chain.tile([P, fd], F32, tag="t_m")
                    if share == 2:
                        t_q = t_m  # in-place chain S3->A2->S5->A3 on t_m
                    else:
                        t_q = chain.tile([P, fd], F32,
                                         tag="t_y" if share else "t_q")
                    t_o = tmp.tile([P, fd], F32,
                                   tag="t_b" if share == 1 else "t_o")

                    # ys = x * inv_a   [ACT]
                    nc.scalar.activation(t_y[:], t_x[:], ACTF.Copy,
                                         bias=0.0, scale=ia_ap)
                    if variant == "safe":
                        # y' = clamp(ys, +-(1-2^-24))   [DVE, in place]
                        nc.vector.tensor_scalar(t_y[:], t_y[:],
                                                CLAMP_HI, -CLAMP_HI,
                                                ALU.min, ALU.max)
                    # base = ys & 0xFF800000   [DVE]
                    nc.vector.tensor_scalar(t_b[:].bitcast(U32),
                                            t_y[:].bitcast(U32),
                                            0xFF800000, None, ALU.bitwise_and)
                    # m = (ys & 0x7FFFFF) | 0x3F800000   [DVE]
                    nc.vector.tensor_scalar(t_m[:].bitcast(U32),
                                            t_y[:].bitcast(U32),
                                            0x007FFFFF, 0x3F800000,
                                            ALU.bitwise_and, ALU.bitwise_or)
                    # r' = (m - 1) * C2: rebalanced DVE/ACT, in place
                    if idx % r_dve_every == 0:
                        nc.vector.tensor_scalar(t_m[:], t_m[:], 1.0, C2,
                                                ALU.subtract, ALU.mult)
                    else:
                        nc.scalar.activation(t_m[:], t_m[:], ACTF.Copy,
                                             bias=-C2, scale=C2)
                    # rq = r' & 0x7F800000   [DVE]
                    nc.vector.tensor_scalar(t_q[:].bitcast(U32),
                                            t_m[:].bitcast(U32),
                                            0x7F800000, None, ALU.bitwise_and)
                    if tail == "stt":
                        # yq = (rq + 1) * base   [DVE fused]
                        nc.vector.scalar_tensor_tensor(
                            t_q[:], t_q[:], 1.0, t_b[:], ALU.add, ALU.mult)
                        # out = yq * a   [ACT]
                        nc.scalar.activation(t_o[:], t_q[:], ACTF.Copy,
                                             bias=0.0, scale=a_ap)
                    else:
                        # mq_a = rq*a + a   [ACT Identity FMA, in place]
                        nc.scalar.activation(t_q[:], t_q[:], ACTF.Identity,
                                             bias=a_ap, scale=a_ap)
                        # out = base * mq_a   [DVE tensor-pair]
                        nc.vector.tensor_tensor(t_o[:], t_b[:], t_q[:],
                                                ALU.mult)

                    nc.sync.dma_start(o_d[rows, cols], t_o[:])
                    idx += 1
    nc.compile()
    return nc


def _get_nc(variant="fast", reps=1, **kw):
    key = (variant, reps, tuple(sorted(kw.items())))
    if key not in _CACHE:
        _CACHE[key] = _build(variant, reps, **kw)
    return _CACHE[key]


def make_consts(alpha):
    a = np.float32(max(float(np.asarray(alpha, dtype=np.float32)), EPS))
    inv_a = np.float32(1.0) / a
    consts = np.zeros((P, 8), np.float32)
    consts[:, 0] = a
    consts[:, 1] = inv_a
    return consts


def kernel(x, alpha, levels=None):
    """Full-input entry point. x: [4096,16384] f32, alpha: scalar f32."""
    from concourse.bass_utils import run_bass_kernel_spmd

    x = np.ascontiguousarray(np.asarray(x, dtype=np.float32))
    a = np.float32(max(float(np.asarray(alpha, dtype=np.float32)), EPS))
    # FAST drops the |ys|<=1 clamp; valid iff no element exceeds alpha.
    variant = "fast" if float(np.abs(x).max()) <= float(a) else "safe"
    consts = make_consts(alpha)

    nc = _get_nc(variant)
    in_maps = [
        {"x": x[i * SHARD_ROWS:(i + 1) * SHARD_ROWS], "consts": consts}
        for i in range(N_CORES)
    ]
    res = run_bass_kernel_spmd(nc, in_maps, core_ids=list(range(N_CORES)))
    out = np.concatenate([res.results[i]["out"] for i in range(N_CORES)],
                         axis=0)
    return out.astype(np.float32)

